# revision 38
# baseline (speedup 1.0000x reference)
"""Batched ragged segment-mean (BERTEmbedder merge loop) on 8 TRN2 NeuronCores.

Strategy
--------
Data-parallel over the batch: each of the 8 cores processes 2 of the 16
sequences (assignment chosen by the host, see below).  Within a sequence,
segment-sum is computed as a block-sparse one-hot matmul on the PE:

    out[t, d] = sum_s onehot[s, t] * x[s, d]

Segment ids are sorted per row, so each 128-subtoken tile only covers a
narrow window of token ids.  The host inspects the ids and builds a static
(s_tile, t_tile) pair schedule: for each 128-wide s-tile we emit matmuls only
into the 128-row t-tiles its ids can touch (union over the sequences that
share the SPMD program slot, so one program serves all 8 cores).

The matmul runs in a single bf16 pass: x is DMA-loaded with an SWDGE
dtype-cast straight into a bf16 tile, so no engine ever makes a data-sized
copy, and the PE streams 1 cycle/row (fp32/fp32r paths measured ~2.2
cycles/row here).  bf16's 8 mantissa bits give ~3e-3 relative error —
inside the 2e-2 output tolerance.

Everything derivable from segment_ids is precomputed on the host (which
must read the ids anyway to build the schedule) and shipped as one small
packed constant input per core: an iota row for the one-hot compare, the
per-partition segment ids (pre-transposed), and per-token reciprocal
counts.  The device program is then just: cast-load x tiles (SWDGE ring) →
one-hot compare (DVE) → accumulate (PE) → scale (ACT/DVE) → store
(Activation HWDGE ring), with loads and stores on separate rings saturating
~400 GB/s of HBM traffic.  The 16 sequences are assigned to the two SPMD
program slots by searching all 6435 8/8 partitions for the one minimizing
total union-schedule pairs.
"""

import os
import numpy as np

B, S, D, T, P = 16, 4096, 768, 2048, 128
NCORES = 8
SPC = B // NCORES          # sequences per core
NST, NTT = S // P, T // P  # 32 s-tiles, 16 t-tiles
DSPLIT = 512               # PSUM bank limit (fp32 words)
SUPER = 4                  # s-tiles per x-load DMA

_cache: dict = {}


def _schedule(segment_ids: np.ndarray):
    """Per program slot q: which t-tiles each s-tile touches, unioned over the
    sequences that run in that slot on every core (SPMD: one program)."""
    from itertools import combinations
    mins = segment_ids.reshape(B, NST, P).min(2) // P
    maxs = segment_ids.reshape(B, NST, P).max(2) // P

    def _npairs(group):
        return int((maxs[list(group)].max(0) - mins[list(group)].min(0) + 1).sum())

    best = None
    allseq = set(range(B))
    for combo in combinations(range(1, B), NCORES - 1):
        g0 = (0,) + combo
        g1 = tuple(sorted(allseq - set(g0)))
        c = _npairs(g0) + _npairs(g1)
        if best is None or c < best[0]:
            best = (c, (g0, g1))
    slot_seqs = best[1]

    sched = []
    for q in range(SPC):
        seqs = list(slot_seqs[q])
        js_of = []
        for i in range(NST):
            blk = segment_ids[seqs, i * P:(i + 1) * P]
            lo, hi = int(blk.min()), int(blk.max())
            js_of.append(list(range(lo // P, hi // P + 1)))
        first, last = {}, {}
        for i in range(NST):
            for j in js_of[i]:
                first.setdefault(j, i)
                last[j] = i
        # loud guard: the PSUM accumulator pools have 4 slots each; more
        # simultaneously-open t-tiles would deadlock the tile scheduler
        maxopen = max(sum(1 for j in first if first[j] <= i <= last[j])
                      for i in range(NST))
        assert maxopen <= 3, f"schedule needs {maxopen} open PSUM accumulators"
        sched.append((tuple(tuple(js) for js in js_of),
                      tuple(sorted(first.items())),
                      tuple(sorted(last.items()))))
    return tuple(sched), slot_seqs


def _maxw(sched):
    return P * max(len(js) for q in range(SPC) for js in sched[q][0])


def _build(sched):
    from contextlib import ExitStack
    import concourse.bacc as bacc
    import concourse.tile as tile
    import concourse.mybir as mybir

    f32, bf16 = mybir.dt.float32, mybir.dt.bfloat16
    AO = mybir.AluOpType
    nc = bacc.Bacc("TRN2", target_bir_lowering=False, debug=False)
    maxw = _maxw(sched)
    # host-packed constants: [iota row | sid_allT (per slot) | invc_T (per slot)]
    HC = maxw + SPC * (NST + NTT)
    x = nc.dram_tensor("raw_output", [SPC, S, D], f32, kind="ExternalInput").ap()
    hc = nc.dram_tensor("hconst", [P, HC], f32, kind="ExternalInput").ap()
    out = nc.dram_tensor("out", [SPC, T, D], f32, kind="ExternalOutput").ap()

    with ExitStack() as ctx:
        # defer stores for t-tiles closing after s-tile CUT: they stage in
        # SBUF and ship after the last x load, so mid-run HBM bandwidth
        # leans toward loads and the tail drains at pure line rate
        CUT = 8
        n_defer = sum(1 for q in range(SPC)
                      for j, l in sched[q][2] if l > CUT)

        tc = ctx.enter_context(tile.TileContext(nc))
        const = ctx.enter_context(tc.tile_pool(name="const", bufs=1))
        xp = ctx.enter_context(tc.tile_pool(name="xp", bufs=10))
        ohp = ctx.enter_context(tc.tile_pool(name="ohp", bufs=24))
        outp = ctx.enter_context(tc.tile_pool(name="outp", bufs=8))
        dfp = ctx.enter_context(tc.tile_pool(name="dfp", bufs=n_defer + 1))
        psa = ctx.enter_context(tc.tile_pool(name="psa", bufs=4, space="PSUM"))
        psb = ctx.enter_context(tc.tile_pool(name="psb", bufs=4, space="PSUM"))

        hct = const.tile([P, HC], f32)
        nc.sync.dma_start(out=hct[:], in_=hc)
        iota_w = hct[:, 0:maxw]

        x_seqs = [x[q].rearrange("(n p) d -> p n d", p=P) for q in range(SPC)]
        ctxs = []
        for q in range(SPC):
            js_of, first_t, last_t = sched[q]
            o = maxw + q * NST
            oi = maxw + SPC * NST + q * NTT
            ctxs.append({
                "js_of": js_of, "first": dict(first_t), "last": dict(last_t),
                "sid_all": hct[:, o:o + NST], "invc_sb": hct[:, oi:oi + NTT],
                "x_seq": x_seqs[q],
                "out_seq": out[q].rearrange("(n p) d -> p n d", p=P),
                "open_ps": {}, "deferred": []})

        def emit_group(q, i0, nsi):
            c = ctxs[q]
            js_of, first, last = c["js_of"], c["first"], c["last"]
            sid_all, open_ps, deferred = c["sid_all"], c["open_ps"], c["deferred"]
            invc_sb = c["invc_sb"]
            xt = xp.tile([P, SUPER, D], bf16, tag="xt", name=f"xt_q{q}_g{i0}")
            nc.gpsimd.dma_start(out=xt[:, 0:nsi, :],
                                in_=c["x_seq"][:, i0:i0 + nsi, :])
            # one-hot windows first: they only depend on host constants, so
            # the DVE can produce them while the x DMA is still in flight
            ohws = []
            for si in range(nsi):
                i = i0 + si
                js = js_of[i]
                ohw = ohp.tile([P, P * len(js)], bf16, tag="oh",
                               name=f"oh_q{q}_i{i}")
                nc.vector.tensor_scalar(
                    ohw[:], iota_w[:, 0:P * len(js)], float(js[0] * P),
                    sid_all[:, i:i + 1], AO.add, AO.is_equal)
                ohws.append(ohw)
            # single bf16 pass at 1 PE cycle per output row
            for si in range(nsi):
                i = i0 + si
                for k, j in enumerate(js_of[i]):
                    st = first[j] == i
                    sp_ = last[j] == i
                    if st:
                        open_ps[j] = (
                            psa.tile([P, DSPLIT], f32, tag="psA",
                                     name=f"accA_q{q}_j{j}"),
                            psb.tile([P, D - DSPLIT], f32, tag="psB",
                                     name=f"accB_q{q}_j{j}"))
                    pa, pb = open_ps[j]
                    oh = ohws[si][:, k * P:(k + 1) * P]
                    nc.tensor.matmul(pa[:], lhsT=oh, rhs=xt[:, si, 0:DSPLIT],
                                     start=st, stop=sp_)
                    nc.tensor.matmul(pb[:], lhsT=oh, rhs=xt[:, si, DSPLIT:D],
                                     start=st, stop=sp_)
                    if sp_:
                        defer = i > CUT
                        pool = dfp if defer else outp
                        ot = pool.tile([P, D], f32, tag="dt" if defer else "ot",
                                       name=f"ot_q{q}_{j}")
                        nc.scalar.activation(ot[:, 0:DSPLIT], pa[:],
                                             mybir.ActivationFunctionType.Copy,
                                             scale=invc_sb[:, j:j + 1])
                        nc.vector.tensor_scalar_mul(
                            ot[:, DSPLIT:D], pb[:], invc_sb[:, j:j + 1])
                        if defer:
                            deferred.append((j, ot))
                        else:
                            nc.scalar.dma_start(
                                out=c["out_seq"][:, j, :], in_=ot[:])
                        del open_ps[j]

        # empty t-tiles are never written: run_bass_via_pjrt donates
        # zero-initialized output buffers (documented contract — "kernels
        # that don't write every element rely on that")

        # interleave the two slots' groups: two independent dependency
        # chains keep every engine fed through the other chain's stalls.
        # All stores ride the SP HWDGE ring, so the Activation engine does
        # nothing but finalize compute (prompt PSUM recycling for the PE).
        taper = [SUPER] * (NST // SUPER)
        assert sum(taper) == NST
        i0 = 0
        starts = [(i0 := i0 + n) - n for n in taper]
        for i0_, n in zip(starts, taper):
            for q in range(SPC):
                emit_group(q, i0_, n)

        # drain: deferred stores ship back-to-back at line rate now that the
        # load stream is finished, in closure (data-ready) order since the
        # SP sequencer triggers them FIFO
        drain = []
        for n in range(max(len(c["deferred"]) for c in ctxs)):
            for c in ctxs:
                if n < len(c["deferred"]):
                    drain.append((c, *c["deferred"][n]))
        for c, j, ot in drain:
            nc.scalar.dma_start(out=c["out_seq"][:, j, :], in_=ot[:])
    nc.compile()
    return nc


def _get_nc(segment_ids: np.ndarray):
    sched, slot_seqs = _schedule(segment_ids)
    if sched not in _cache:
        _cache[sched] = _build(sched)
    return _cache[sched], sched, slot_seqs


def run(raw_output, segment_ids, trace=False):
    from concourse.bass_utils import run_bass_kernel_spmd

    raw_output = np.ascontiguousarray(raw_output, dtype=np.float32)
    segment_ids = np.ascontiguousarray(segment_ids, dtype=np.int32)
    nc, sched, slot_seqs = _get_nc(segment_ids)
    maxw = _maxw(sched)
    cnts = np.stack([np.bincount(segment_ids[b], minlength=T) for b in range(B)])
    inv_cnt = (1.0 / np.maximum(cnts, 1)).astype(np.float32)
    iota_row = np.broadcast_to(np.arange(maxw, dtype=np.float32), (P, maxw))
    in_maps = []
    for c in range(NCORES):
        seqs = [slot_seqs[q][c] for q in range(SPC)]
        hconst = np.concatenate(
            [iota_row]
            + [segment_ids[s].reshape(NST, P).T.astype(np.float32) for s in seqs]
            + [inv_cnt[s].reshape(NTT, P).T for s in seqs], axis=1)
        in_maps.append({
            "raw_output": np.ascontiguousarray(raw_output[seqs]),
            "hconst": np.ascontiguousarray(hconst)})
    bkr = run_bass_kernel_spmd(nc, in_maps, list(range(NCORES)), trace=trace)
    full = np.empty((B, T, D), np.float32)
    for c in range(NCORES):
        for q in range(SPC):
            full[slot_seqs[q][c]] = bkr.results[c]["out"][q]
    return full, bkr


def kernel(raw_output, segment_ids):
    full, _ = run(raw_output, segment_ids,
                  trace=bool(int(os.environ.get("KERNEL_TRACE", "0"))))
    return full


# revision 39
# speedup vs baseline: 1.0749x; 1.0749x over previous
"""Batched ragged segment-mean (BERTEmbedder merge loop) on 8 TRN2 NeuronCores.

Strategy
--------
Data-parallel over the batch: each of the 8 cores processes 2 of the 16
sequences (assignment chosen by the host, see below).  Within a sequence,
segment-sum is computed as a block-sparse one-hot matmul on the PE:

    out[t, d] = sum_s onehot[s, t] * x[s, d]

Segment ids are sorted per row, so each 128-subtoken tile only covers a
narrow window of token ids.  The host inspects the ids and builds a static
(s_tile, t_tile) pair schedule: for each 128-wide s-tile we emit matmuls only
into the 128-row t-tiles its ids can touch (union over the sequences that
share the SPMD program slot, so one program serves all 8 cores).

The matmul runs in a single bf16 pass: x is DMA-loaded with an SWDGE
dtype-cast straight into a bf16 tile, so no engine ever makes a data-sized
copy, and the PE streams 1 cycle/row (fp32/fp32r paths measured ~2.2
cycles/row here).  bf16's 8 mantissa bits give ~3e-3 relative error —
inside the 2e-2 output tolerance.

Everything derivable from segment_ids is precomputed on the host (which
must read the ids anyway to build the schedule) and shipped as one small
packed constant input per core: an iota row for the one-hot compare, the
per-partition segment ids (pre-transposed), and per-token reciprocal
counts.  The device program is then just: cast-load x tiles (SWDGE ring) →
one-hot compare (DVE) → accumulate (PE) → scale (ACT/DVE) → store
(Activation HWDGE ring), with loads and stores on separate rings saturating
~400 GB/s of HBM traffic.  Stores for t-tiles that close late are staged in
SBUF and drained after the last load so mid-run bandwidth leans toward
loads and the tail runs at pure store line rate; all-empty t-tiles are
never written at all (run_bass_via_pjrt donates zero-initialized output
buffers).  The 16 sequences are assigned to the two SPMD program slots by
searching all 6435 8/8 partitions for the one minimizing total
union-schedule pairs.
"""

import os
import numpy as np

B, S, D, T, P = 16, 4096, 768, 2048, 128
NCORES = 8
SPC = B // NCORES          # sequences per core
NST, NTT = S // P, T // P  # 32 s-tiles, 16 t-tiles
DSPLIT = 512               # PSUM bank limit (fp32 words)
SUPER = 4                  # s-tiles per x-load DMA

_cache: dict = {}


def _schedule(segment_ids: np.ndarray):
    """Per program slot q: which t-tiles each s-tile touches, unioned over the
    sequences that run in that slot on every core (SPMD: one program)."""
    from itertools import combinations
    mins = segment_ids.reshape(B, NST, P).min(2) // P
    maxs = segment_ids.reshape(B, NST, P).max(2) // P

    def _npairs(group):
        return int((maxs[list(group)].max(0) - mins[list(group)].min(0) + 1).sum())

    best = None
    allseq = set(range(B))
    for combo in combinations(range(1, B), NCORES - 1):
        g0 = (0,) + combo
        g1 = tuple(sorted(allseq - set(g0)))
        c = _npairs(g0) + _npairs(g1)
        if best is None or c < best[0]:
            best = (c, (g0, g1))
    slot_seqs = best[1]

    sched = []
    for q in range(SPC):
        seqs = list(slot_seqs[q])
        js_of = []
        for i in range(NST):
            blk = segment_ids[seqs, i * P:(i + 1) * P]
            lo, hi = int(blk.min()), int(blk.max())
            js_of.append(list(range(lo // P, hi // P + 1)))
        first, last = {}, {}
        for i in range(NST):
            for j in js_of[i]:
                first.setdefault(j, i)
                last[j] = i
        # loud guard: the PSUM accumulator pools have 4 slots each; more
        # simultaneously-open t-tiles would deadlock the tile scheduler
        maxopen = max(sum(1 for j in first if first[j] <= i <= last[j])
                      for i in range(NST))
        assert maxopen <= 3, f"schedule needs {maxopen} open PSUM accumulators"
        sched.append((tuple(tuple(js) for js in js_of),
                      tuple(sorted(first.items())),
                      tuple(sorted(last.items()))))
    return tuple(sched), slot_seqs


def _maxw(sched):
    return P * max(len(js) for q in range(SPC) for js in sched[q][0])


def _build(sched):
    from contextlib import ExitStack
    import concourse.bacc as bacc
    import concourse.tile as tile
    import concourse.mybir as mybir

    f32, bf16 = mybir.dt.float32, mybir.dt.bfloat16
    AO = mybir.AluOpType
    nc = bacc.Bacc("TRN2", target_bir_lowering=False, debug=False)
    maxw = _maxw(sched)
    # host-packed constants: [iota row | sid_allT (per slot) | invc_T (per slot)]
    HC = maxw + SPC * (NST + NTT)
    x = nc.dram_tensor("raw_output", [SPC, S, D], f32, kind="ExternalInput").ap()
    hc = nc.dram_tensor("hconst", [P, HC], f32, kind="ExternalInput").ap()
    out = nc.dram_tensor("out", [SPC, T, D], f32, kind="ExternalOutput").ap()

    with ExitStack() as ctx:
        # defer stores for t-tiles closing after s-tile CUT: they stage in
        # SBUF and ship after the last x load, so mid-run HBM bandwidth
        # leans toward loads and the tail drains at pure line rate
        CUT = 8
        n_defer = sum(1 for q in range(SPC)
                      for j, l in sched[q][2] if l > CUT)

        tc = ctx.enter_context(tile.TileContext(nc))
        const = ctx.enter_context(tc.tile_pool(name="const", bufs=1))
        xp = ctx.enter_context(tc.tile_pool(name="xp", bufs=10))
        ohp = ctx.enter_context(tc.tile_pool(name="ohp", bufs=24))
        outp = ctx.enter_context(tc.tile_pool(name="outp", bufs=8))
        dfp = ctx.enter_context(tc.tile_pool(name="dfp", bufs=n_defer + 1))
        psa = ctx.enter_context(tc.tile_pool(name="psa", bufs=4, space="PSUM"))
        psb = ctx.enter_context(tc.tile_pool(name="psb", bufs=4, space="PSUM"))

        hct = const.tile([P, HC], f32)
        nc.sync.dma_start(out=hct[:], in_=hc)
        iota_w = hct[:, 0:maxw]

        x_seqs = [x[q].rearrange("(n p) d -> p n d", p=P) for q in range(SPC)]
        ctxs = []
        for q in range(SPC):
            js_of, first_t, last_t = sched[q]
            o = maxw + q * NST
            oi = maxw + SPC * NST + q * NTT
            ctxs.append({
                "js_of": js_of, "first": dict(first_t), "last": dict(last_t),
                "sid_all": hct[:, o:o + NST], "invc_sb": hct[:, oi:oi + NTT],
                "x_seq": x_seqs[q],
                "out_seq": out[q].rearrange("(n p) d -> p n d", p=P),
                "open_ps": {}, "deferred": []})

        def emit_group(q, i0, nsi):
            c = ctxs[q]
            js_of, first, last = c["js_of"], c["first"], c["last"]
            sid_all, open_ps, deferred = c["sid_all"], c["open_ps"], c["deferred"]
            invc_sb = c["invc_sb"]
            xt = xp.tile([P, SUPER, D], bf16, tag="xt", name=f"xt_q{q}_g{i0}")
            nc.gpsimd.dma_start(out=xt[:, 0:nsi, :],
                                in_=c["x_seq"][:, i0:i0 + nsi, :])
            # one-hot windows first: they only depend on host constants, so
            # the DVE can produce them while the x DMA is still in flight
            ohws = []
            for si in range(nsi):
                i = i0 + si
                js = js_of[i]
                ohw = ohp.tile([P, P * len(js)], bf16, tag="oh",
                               name=f"oh_q{q}_i{i}")
                nc.vector.tensor_scalar(
                    ohw[:], iota_w[:, 0:P * len(js)], float(js[0] * P),
                    sid_all[:, i:i + 1], AO.add, AO.is_equal)
                ohws.append(ohw)
            # single bf16 pass at 1 PE cycle per output row
            for si in range(nsi):
                i = i0 + si
                for k, j in enumerate(js_of[i]):
                    st = first[j] == i
                    sp_ = last[j] == i
                    if st:
                        open_ps[j] = (
                            psa.tile([P, DSPLIT], f32, tag="psA",
                                     name=f"accA_q{q}_j{j}"),
                            psb.tile([P, D - DSPLIT], f32, tag="psB",
                                     name=f"accB_q{q}_j{j}"))
                    pa, pb = open_ps[j]
                    oh = ohws[si][:, k * P:(k + 1) * P]
                    nc.tensor.matmul(pa[:], lhsT=oh, rhs=xt[:, si, 0:DSPLIT],
                                     start=st, stop=sp_)
                    nc.tensor.matmul(pb[:], lhsT=oh, rhs=xt[:, si, DSPLIT:D],
                                     start=st, stop=sp_)
                    if sp_:
                        defer = i > CUT
                        pool = dfp if defer else outp
                        ot = pool.tile([P, D], f32, tag="dt" if defer else "ot",
                                       name=f"ot_q{q}_{j}")
                        nc.scalar.activation(ot[:, 0:DSPLIT], pa[:],
                                             mybir.ActivationFunctionType.Copy,
                                             scale=invc_sb[:, j:j + 1])
                        nc.vector.tensor_scalar_mul(
                            ot[:, DSPLIT:D], pb[:], invc_sb[:, j:j + 1])
                        if defer:
                            deferred.append((j, ot))
                        else:
                            nc.scalar.dma_start(
                                out=c["out_seq"][:, j, :], in_=ot[:])
                        del open_ps[j]

        # empty t-tiles are never written: run_bass_via_pjrt donates
        # zero-initialized output buffers (documented contract — "kernels
        # that don't write every element rely on that")

        # interleave the two slots' groups: two independent dependency
        # chains keep every engine fed through the other chain's stalls.
        # All stores ride the SP HWDGE ring, so the Activation engine does
        # nothing but finalize compute (prompt PSUM recycling for the PE).
        taper = [SUPER] * (NST // SUPER)
        assert sum(taper) == NST
        i0 = 0
        starts = [(i0 := i0 + n) - n for n in taper]
        for i0_, n in zip(starts, taper):
            for q in range(SPC):
                emit_group(q, i0_, n)

        # drain: deferred stores ship back-to-back at line rate now that the
        # load stream is finished, in closure (data-ready) order since the
        # SP sequencer triggers them FIFO
        drain = []
        for n in range(max(len(c["deferred"]) for c in ctxs)):
            for c in ctxs:
                if n < len(c["deferred"]):
                    drain.append((c, *c["deferred"][n]))
        for c, j, ot in drain:
            nc.scalar.dma_start(out=c["out_seq"][:, j, :], in_=ot[:])
    nc.compile()
    return nc


def _get_nc(segment_ids: np.ndarray):
    sched, slot_seqs = _schedule(segment_ids)
    if sched not in _cache:
        _cache[sched] = _build(sched)
    return _cache[sched], sched, slot_seqs


def run(raw_output, segment_ids, trace=False):
    from concourse.bass_utils import run_bass_kernel_spmd

    raw_output = np.ascontiguousarray(raw_output, dtype=np.float32)
    segment_ids = np.ascontiguousarray(segment_ids, dtype=np.int32)
    nc, sched, slot_seqs = _get_nc(segment_ids)
    maxw = _maxw(sched)
    cnts = np.stack([np.bincount(segment_ids[b], minlength=T) for b in range(B)])
    inv_cnt = (1.0 / np.maximum(cnts, 1)).astype(np.float32)
    iota_row = np.broadcast_to(np.arange(maxw, dtype=np.float32), (P, maxw))
    in_maps = []
    for c in range(NCORES):
        seqs = [slot_seqs[q][c] for q in range(SPC)]
        hconst = np.concatenate(
            [iota_row]
            + [segment_ids[s].reshape(NST, P).T.astype(np.float32) for s in seqs]
            + [inv_cnt[s].reshape(NTT, P).T for s in seqs], axis=1)
        in_maps.append({
            "raw_output": np.ascontiguousarray(raw_output[seqs]),
            "hconst": np.ascontiguousarray(hconst)})
    bkr = run_bass_kernel_spmd(nc, in_maps, list(range(NCORES)), trace=trace)
    full = np.empty((B, T, D), np.float32)
    for c in range(NCORES):
        for q in range(SPC):
            full[slot_seqs[q][c]] = bkr.results[c]["out"][q]
    return full, bkr


def kernel(raw_output, segment_ids):
    full, _ = run(raw_output, segment_ids,
                  trace=bool(int(os.environ.get("KERNEL_TRACE", "0"))))
    return full
